# revision 1
# baseline (speedup 1.0000x reference)
"""CenterLoss Trainium2 kernel.

loss = mean_i ||x[i] - centers[labels[i]]||^2

The one-hot-masked distance matrix in the reference collapses to a row
gather of `centers`, so the kernel is a gather + fused square-reduce:
data-parallel over 8 NeuronCores (512 batch rows each, centers
replicated), with the final 8-way scalar all-reduce done on host.
"""

import os
import sys

import numpy as np

for _p in ("/opt/trn_rl_repo", "/root/.axon_site/_ro/trn_rl_repo", "/root/.axon_site", "/root/.axon_site/_ro/pypackages"):
    if os.path.isdir(_p) and _p not in sys.path:
        sys.path.append(_p)

NCORES = 8
B = 4096
D = 128
C = 50000
P = 128
B_LOC = B // NCORES          # 512 rows per core
NTILES = B_LOC // P          # 4 row-tiles of 128

_cached = None


def _build():
    import concourse.bacc as bacc
    import concourse.bass as bass
    import concourse.mybir as mybir
    import concourse.tile as tile

    nc = bacc.Bacc(
        "TRN2",
        target_bir_lowering=False,
        debug=False,
        enable_asserts=True,
        num_devices=NCORES,
    )
    x = nc.dram_tensor("x", [B_LOC, D], mybir.dt.float32, kind="ExternalInput").ap()
    labels = nc.dram_tensor("labels", [P, NTILES], mybir.dt.int32, kind="ExternalInput").ap()
    centers = nc.dram_tensor("centers", [C, D], mybir.dt.float32, kind="ExternalInput").ap()
    out = nc.dram_tensor("out", [1, 1], mybir.dt.float32, kind="ExternalOutput").ap()

    # x[n*P + p, d] -> partition p, free column n*D + d
    x_src = x.rearrange("(n p) d -> p n d", p=P)

    with tile.TileContext(nc) as tc:
        with (
            tc.tile_pool(name="sbuf", bufs=1) as pool,
            tc.tile_pool(name="psum", bufs=1, space="PSUM") as psum_pool,
        ):
            x_all = pool.tile([P, NTILES * D], mybir.dt.float32)
            c_all = pool.tile([P, NTILES * D], mybir.dt.float32)
            idx_all = pool.tile([P, NTILES], mybir.dt.int32)
            ones = pool.tile([P, 1], mybir.dt.float32)
            col = pool.tile([P, 1], mybir.dt.float32)
            sq = pool.tile([P, NTILES * D], mybir.dt.float32)
            res = pool.tile([1, 1], mybir.dt.float32)

            nc.vector.memset(ones[:], 1.0)
            nc.sync.dma_start(out=x_all[:].rearrange("p (n d) -> p n d", d=D), in_=x_src)
            nc.sync.dma_start(out=idx_all[:], in_=labels[:])
            for i in range(NTILES):
                nc.gpsimd.indirect_dma_start(
                    out=c_all[:, i * D : (i + 1) * D],
                    out_offset=None,
                    in_=centers[:],
                    in_offset=bass.IndirectOffsetOnAxis(ap=idx_all[:, i : i + 1], axis=0),
                )
            nc.vector.tensor_tensor(
                out=sq[:], in0=x_all[:], in1=c_all[:], op=mybir.AluOpType.subtract
            )
            # Square with per-partition accumulate: col[p] = sum_f sq[p,f]^2
            nc.scalar.activation(
                out=sq[:],
                in_=sq[:],
                func=mybir.ActivationFunctionType.Square,
                accum_out=col[:],
            )
            ps = psum_pool.tile([1, 1], mybir.dt.float32)
            nc.tensor.matmul(out=ps[:], lhsT=col[:], rhs=ones[:], start=True, stop=True)
            nc.scalar.mul(res[:], ps[:], 1.0 / B)
            nc.sync.dma_start(out=out[:], in_=res[:])

    nc.compile()
    return nc


def _get_nc():
    global _cached
    if _cached is None:
        _cached = _build()
    return _cached


def kernel(x, labels, centers, **profile_kwargs):
    from concourse.bass_utils import run_bass_kernel_spmd

    nc = _get_nc()
    x = np.ascontiguousarray(np.asarray(x), dtype=np.float32)
    centers = np.ascontiguousarray(np.asarray(centers), dtype=np.float32)
    labels32 = np.asarray(labels).astype(np.int32)

    in_maps = []
    for k in range(NCORES):
        xs = x[k * B_LOC : (k + 1) * B_LOC]
        # labels packed so partition p, column n holds label of row n*P + p
        ls = np.ascontiguousarray(
            labels32[k * B_LOC : (k + 1) * B_LOC].reshape(NTILES, P).T
        )
        in_maps.append({"x": xs, "labels": ls, "centers": centers})

    r = run_bass_kernel_spmd(nc, in_maps, core_ids=list(range(NCORES)), **profile_kwargs)
    total = sum(float(m["out"][0, 0]) for m in r.results)
    out = np.array(total, dtype=np.float32)
    if profile_kwargs:
        return out, r
    return out


# revision 3
# speedup vs baseline: 1.3128x; 1.3128x over previous
"""CenterLoss Trainium2 kernel.

loss = mean_i ||x[i] - centers[labels[i]]||^2

The one-hot-masked distance matrix in the reference collapses to a row
gather of `centers`, so the kernel is a gather + fused square-reduce:
data-parallel over 8 NeuronCores (512 batch rows each, centers
replicated), with the final 8-way scalar all-reduce done on host.
"""

import os
import sys

import numpy as np

for _p in ("/opt/trn_rl_repo", "/root/.axon_site/_ro/trn_rl_repo", "/root/.axon_site", "/root/.axon_site/_ro/pypackages"):
    if os.path.isdir(_p) and _p not in sys.path:
        sys.path.append(_p)

NCORES = 8
B = 4096
D = 128
C = 50000
P = 128
B_LOC = B // NCORES          # 512 rows per core
NTILES = B_LOC // P          # 4 row-tiles of 128

_cached = None


def _build():
    import concourse.bacc as bacc
    import concourse.bass as bass
    import concourse.mybir as mybir
    import concourse.tile as tile

    nc = bacc.Bacc(
        "TRN2",
        target_bir_lowering=False,
        debug=False,
        enable_asserts=False,
        num_devices=NCORES,
    )
    x = nc.dram_tensor("x", [B_LOC, D], mybir.dt.float32, kind="ExternalInput").ap()
    labels = nc.dram_tensor("labels", [P, NTILES], mybir.dt.int32, kind="ExternalInput").ap()
    centers = nc.dram_tensor("centers", [C, D], mybir.dt.float32, kind="ExternalInput").ap()
    out = nc.dram_tensor("out", [1, 1], mybir.dt.float32, kind="ExternalOutput").ap()

    # x[n*P + p, d] -> partition p, free column n*D + d
    x_src = x.rearrange("(n p) d -> p n d", p=P)

    with tile.TileContext(nc) as tc:
        with (
            tc.tile_pool(name="sbuf", bufs=1) as pool,
            tc.tile_pool(name="psum", bufs=1, space="PSUM") as psum_pool,
        ):
            x_all = pool.tile([P, NTILES * D], mybir.dt.float32)
            c_all = pool.tile([P, NTILES * D], mybir.dt.float32)
            idx_all = pool.tile([P, NTILES], mybir.dt.int32)
            ones = pool.tile([P, 1], mybir.dt.float32)
            col = pool.tile([P, 1], mybir.dt.float32)
            sq = pool.tile([P, NTILES * D], mybir.dt.float32)
            res = pool.tile([1, 1], mybir.dt.float32)

            nc.vector.memset(ones[:], 1.0)
            # labels first — the gather is the critical path
            nc.sync.dma_start(out=idx_all[:], in_=labels[:])
            # one fused gather: index [p, n] -> c_all[p, n*D:(n+1)*D]
            nc.gpsimd.indirect_dma_start(
                out=c_all[:],
                out_offset=None,
                in_=centers[:],
                in_offset=bass.IndirectOffsetOnAxis(ap=idx_all[:], axis=0),
            )
            # x goes on the ACT HWDGE ring, not needed until the subtract
            nc.scalar.dma_start(out=x_all[:].rearrange("p (n d) -> p n d", d=D), in_=x_src)
            nc.vector.tensor_tensor(
                out=sq[:], in0=x_all[:], in1=c_all[:], op=mybir.AluOpType.subtract
            )
            # Square with per-partition accumulate: col[p] = sum_f sq[p,f]^2
            nc.scalar.activation(
                out=sq[:],
                in_=sq[:],
                func=mybir.ActivationFunctionType.Square,
                accum_out=col[:],
            )
            ps = psum_pool.tile([1, 1], mybir.dt.float32)
            nc.tensor.matmul(out=ps[:], lhsT=col[:], rhs=ones[:], start=True, stop=True)
            nc.scalar.mul(res[:], ps[:], 1.0 / B)
            nc.sync.dma_start(out=out[:], in_=res[:])

    nc.compile()
    return nc


def _get_nc():
    global _cached
    if _cached is None:
        _cached = _build()
    return _cached


def kernel(x, labels, centers, **profile_kwargs):
    from concourse.bass_utils import run_bass_kernel_spmd

    nc = _get_nc()
    x = np.ascontiguousarray(np.asarray(x), dtype=np.float32)
    centers = np.ascontiguousarray(np.asarray(centers), dtype=np.float32)
    labels32 = np.asarray(labels).astype(np.int32)

    in_maps = []
    for k in range(NCORES):
        xs = x[k * B_LOC : (k + 1) * B_LOC]
        # labels packed so partition p, column n holds label of row n*P + p
        ls = np.ascontiguousarray(
            labels32[k * B_LOC : (k + 1) * B_LOC].reshape(NTILES, P).T
        )
        in_maps.append({"x": xs, "labels": ls, "centers": centers})

    r = run_bass_kernel_spmd(nc, in_maps, core_ids=list(range(NCORES)), **profile_kwargs)
    total = sum(float(m["out"][0, 0]) for m in r.results)
    out = np.array(total, dtype=np.float32)
    if profile_kwargs:
        return out, r
    return out
